# revision 51
# baseline (speedup 1.0000x reference)
"""GQA attention block (rmsnorm + qk-headnorm + rotary + softmax + out-proj)
for Trainium2, SPMD over 8 NeuronCores: 2-way data parallel (batch) x 4-way
tensor parallel (query-head groups). Partial out-proj results are summed on
host (the TP all-reduce).

Shapes (hardcoded): tokens [2,2048,2048] f32, rotary [2048,64], norm_w [2048],
Wq [2048,2048], Wkv [2048,1024], Wo [2048,2048], gamma_q [32,64], gamma_k [8,64].

v2 design notes (cost-model driven):
- tokens sent f16; x^T built by the DMA xbar transpose engine (no PE transposes)
- rmsnorm rstd cancels for Q/K (scale-invariant l2 headnorm); folded into V only
- K-side headnorm folded into kr via contract-1 matmul broadcasts + DVE muls
- AV matmul runs with exp-tile stationary / V moving (65 cols vs 512): half the
  PE cycles of the q-moving orientation; softmax denominator is V's 65th column
  and normalization is a per-partition tensor_scalar on DVE (no PE broadcasts)
- attention emission interleaves one projection/out-proj matmul per chunk so
  the in-order PE queue has filler during each exp wait
- out-projection drains through DVE copies; output DMA'd as f16 partials
"""

import sys

for _p in ("/opt/trn_rl_repo", "/opt/pypackages"):
    if _p not in sys.path:
        sys.path.append(_p)

from contextlib import ExitStack

import numpy as np

import concourse.bass as bass
import concourse.tile as tile
from concourse import bacc, mybir
from concourse.bass_utils import run_bass_kernel_spmd

B, N, DIM = 2, 2048, 2048
DH = 64  # head dim
QH, KVH = 32, 8
NCORES = 8
TPG = 4  # tensor-parallel groups
QH_PER = QH // TPG  # 8 q heads per core
NPAIR = QH_PER // 2  # 4 pairs of q heads packed 2-per-128-partitions
NT = N // 128  # 16 token tiles
NC = DIM // 128  # 16 contraction chunks
STRIP = 512  # q-strip width in attention
NSTRIP = N // STRIP

F32 = mybir.dt.float32
F32R = mybir.dt.float32r
F16 = mybir.dt.float16
AF = mybir.ActivationFunctionType
ALU = mybir.AluOpType

_BUILD = {}


def _build_nc():
    """Trace + compile the per-core Bass kernel (same program all cores)."""
    nc = bacc.Bacc(
        "TRN2", target_bir_lowering=False, debug=False, num_devices=NCORES
    )

    t_tokens = nc.dram_tensor("tokens", [N, DIM], F16, kind="ExternalInput").ap()
    t_wq = nc.dram_tensor("wq", [DIM, QH_PER * DH], F16, kind="ExternalInput").ap()
    t_wk = nc.dram_tensor("wk", [DIM, 2 * DH], F16, kind="ExternalInput").ap()
    t_wv = nc.dram_tensor("wv", [DIM, 2 * DH], F16, kind="ExternalInput").ap()
    t_wo = nc.dram_tensor("wo", [QH_PER * DH, DIM], F16, kind="ExternalInput").ap()
    t_selq = nc.dram_tensor(
        "selq", [128, 2 * NPAIR], F16, kind="ExternalInput"
    ).ap()
    t_selk = nc.dram_tensor("selk", [128, 2], F16, kind="ExternalInput").ap()
    t_bc2 = nc.dram_tensor("bc2", [2, 128], F32R, kind="ExternalInput").ap()
    t_cos = nc.dram_tensor("cos_t", [128, N], F16, kind="ExternalInput").ap()
    t_sin = nc.dram_tensor("sins_t", [128, N], F16, kind="ExternalInput").ap()
    t_out = nc.dram_tensor("out", [N, DIM], F16, kind="ExternalOutput").ap()

    with tile.TileContext(nc) as tc, ExitStack() as ctx:
        # ---------------- persistent pools (whole kernel) ----------------
        persist = ctx.enter_context(tc.tile_pool(name="persist", bufs=1))
        qr_pool = ctx.enter_context(tc.tile_pool(name="qrp", bufs=1))

        eps_t = persist.tile([128, 1], F32)
        nc.vector.memset(eps_t, float(np.finfo(np.float32).eps))
        tiny_t = persist.tile([128, 1], F32)
        nc.vector.memset(tiny_t, 1e-24)
        # rotated q (unit-norm) / k (norm 8) in f16, [2 heads * 64d, ntok]
        qr_t = [
            qr_pool.tile([128, N], F16, tag=f"qr{p}", name=f"qr{p}")
            for p in range(NPAIR)
        ]
        kr_t = qr_pool.tile([128, N], F16, tag="kr")
        # V (rmsnorm folded), natural layout + ones col: [tok, tile, head, 65]
        v_all = qr_pool.tile([128, NT, 2, DH + 1], F16, tag="vall")
        nc.vector.memset(v_all[:, :, :, DH : DH + 1], 1.0)
        # transposed attention outputs per pair: [2 heads * 64d, ntok]
        o_t = [
            qr_pool.tile([128, N], F16, tag=f"o{p}", name=f"o{p}")
            for p in range(NPAIR)
        ]

        # ---- PSUM pools: 4 (sps) + 2 (av) + 1 (auxQ) + 1 (auxH) = 8 banks --
        sps_pool = ctx.enter_context(
            tc.tile_pool(name="sps", bufs=2, space="PSUM")
        )
        av_pool = ctx.enter_context(
            tc.tile_pool(name="avp", bufs=1, space="PSUM")
        )
        auxq_pool = ctx.enter_context(
            tc.tile_pool(name="auxq", bufs=1, space="PSUM")
        )
        auxh_pool = ctx.enter_context(
            tc.tile_pool(name="auxh", bufs=1, space="PSUM")
        )
        # exp tiles ring
        epool = ctx.enter_context(tc.tile_pool(name="epool", bufs=3))
        dpool = ctx.enter_context(tc.tile_pool(name="dpool", bufs=2))
        onat_pool = ctx.enter_context(tc.tile_pool(name="onatp", bufs=8))

        with ExitStack() as aa:
            wpool = aa.enter_context(tc.tile_pool(name="wpool", bufs=1))
            xt_pool = aa.enter_context(tc.tile_pool(name="xtp", bufs=1))
            xa_pool = aa.enter_context(tc.tile_pool(name="xap", bufs=1))
            spool = aa.enter_context(tc.tile_pool(name="spool", bufs=3))
            cpool = aa.enter_context(tc.tile_pool(name="cpool", bufs=1))
            hpool = aa.enter_context(tc.tile_pool(name="hpool", bufs=2))

            wk_sb = wpool.tile([128, NC, 2 * DH], F16)
            nc.sync.dma_start(
                out=wk_sb[:], in_=t_wk.rearrange("(c p) q -> p c q", p=128)
            )
            wv_sb = wpool.tile([128, NC, 2 * DH], F16)
            nc.sync.dma_start(
                out=wv_sb[:], in_=t_wv.rearrange("(c p) q -> p c q", p=128)
            )
            selk_sb = wpool.tile([128, 2], F16)
            nc.sync.dma_start(out=selk_sb[:], in_=t_selk)
            selq_sb = wpool.tile([128, 2 * NPAIR], F16)
            nc.sync.dma_start(out=selq_sb[:], in_=t_selq)
            bc2_sb = wpool.tile([2, 128], F32R)
            nc.sync.dma_start(out=bc2_sb[:], in_=t_bc2)
            cos_sb = wpool.tile([128, N], F16)
            nc.sync.dma_start(out=cos_sb[:], in_=t_cos)
            sin_sb = wpool.tile([128, N], F16)
            nc.sync.dma_start(out=sin_sb[:], in_=t_sin)
            wq_sb = wpool.tile([128, NC, QH_PER * DH], F16)

            # x^T, f16, per 512-token strip: [dim-in-chunk-part, chunk, 512]
            xt_s = [
                xt_pool.tile([128, NC, 512], F16, tag=f"xt{s}", name=f"xt{s}")
                for s in range(4)
            ]
            # per-token 1/rms for the V projection, col per token tile
            rstd_all = xa_pool.tile([128, NT], F32, tag="rstd")
            # ln(||k||^2/64) per head-half; exp(-0.5 ln) is applied after
            # the broadcast so ACT stays on one {ln, exp} table
            hkl2 = xa_pool.tile([2, N], F32R, tag="hkl2")
            sq_scr = xa_pool.tile([128, DIM], F16, tag="sqscr")

            def a_tile(tt):
                """Load token tile tt, ssum->rstd, xbar-transpose into xt_s."""
                x_nat = xa_pool.tile([128, DIM], F16, tag="xnat", bufs=2)
                nc.sync.dma_start(
                    out=x_nat[:], in_=t_tokens[tt * 128 : (tt + 1) * 128, :]
                )
                ssum = spool.tile([128, 1], F32, tag="ssum")
                nc.vector.tensor_mul(sq_scr[:], x_nat[:], x_nat[:])
                nc.vector.tensor_reduce(
                    ssum[:], sq_scr[:], mybir.AxisListType.X, ALU.add
                )
                rs = spool.tile([128, 1], F32, tag="rs")
                nc.scalar.activation(
                    out=rs[:],
                    in_=ssum[:],
                    func=AF.Ln,
                    bias=eps_t[:],
                    scale=1.0 / DIM,
                )
                nc.scalar.activation(
                    out=rstd_all[:, tt : tt + 1],
                    in_=rs[:],
                    func=AF.Exp,
                    bias=0.0,
                    scale=-0.5,
                )
                st, off = tt // 4, (tt % 4) * 128
                nc.sync.dma_start_transpose(
                    xt_s[st][:, :, off : off + 128], x_nat[:]
                )

            def v_tile(tb):
                vps = auxq_pool.tile([128, 2, DH], F32, tag="auxq", name="vps")
                for c in range(NC):
                    nc.tensor.matmul(
                        vps[:],
                        xt_s[tb // 4][:, c, (tb % 4) * 128 : (tb % 4 + 1) * 128],
                        wv_sb[:, c, :],
                        start=(c == 0),
                        stop=(c == NC - 1),
                    )
                # fold rmsnorm rstd into V (per-token = per-partition here)
                nc.vector.tensor_scalar_mul(
                    v_all[:, tb, :, 0:DH],
                    vps[:],
                    rstd_all[:, tb : tb + 1],
                )

            def k_slice(s):
                """Project K slice s; stash raw k' and 8/||k|| factors."""
                ssl = slice(s * 512, (s + 1) * 512)
                kps = auxq_pool.tile([128, 512], F32, tag="auxq", name="kps")
                for c in range(NC):
                    nc.tensor.matmul(
                        kps[:],
                        wk_sb[:, c, :],
                        xt_s[s][:, c, :],
                        start=(c == 0),
                        stop=(c == NC - 1),
                    )
                nc.vector.tensor_copy(qn_k[:, ssl], kps[:])
                sq2 = hpool.tile([128, 512], F16, tag="sq2")
                nc.vector.tensor_mul(sq2[:], qn_k[:, ssl], qn_k[:, ssl])
                hsum = auxh_pool.tile([2, 512], F32, tag="auxh", name="hsum")
                nc.tensor.matmul(
                    hsum[:], selk_sb[:], sq2[:], start=True, stop=True
                )
                # ln(||k||^2/64); exp(-0.5 ln) later gives 8/||k||
                nc.scalar.activation(
                    out=hkl2[:, ssl],
                    in_=hsum[:],
                    func=AF.Ln,
                    bias=tiny_t[0:2],
                    scale=1.0 / DH,
                )

            def q_slice(p, s, qn):
                """Project q pair p slice s into qn (unit-norm rows)."""
                ssl = slice(s * 512, (s + 1) * 512)
                qps = auxq_pool.tile(
                    [128, 512], F32, tag="auxq", name="qps"
                )
                for c in range(NC):
                    nc.tensor.matmul(
                        qps[:],
                        wq_sb[:, c, p * 128 : (p + 1) * 128],
                        xt_s[s][:, c, :],
                        start=(c == 0),
                        stop=(c == NC - 1),
                    )
                q_slice_norm(p, s, qn, qps)

            def q_slice_norm(p, s, qn, qps):
                ssl = slice(s * 512, (s + 1) * 512)
                q_raw = hpool.tile([128, 512], F16, tag="qraw")
                nc.vector.tensor_copy(q_raw[:], qps[:])
                sq2 = hpool.tile([128, 512], F16, tag="sq2")
                nc.vector.tensor_mul(sq2[:], q_raw[:], q_raw[:])
                hsum = auxh_pool.tile([2, 512], F32, tag="auxh", name="hsum")
                nc.tensor.matmul(
                    hsum[:],
                    selq_sb[:, 2 * p : 2 * p + 2],
                    sq2[:],
                    start=True,
                    stop=True,
                )
                hl2 = hpool.tile([2, 512], F32R, tag="hsq")
                nc.scalar.activation(
                    out=hl2[:],
                    in_=hsum[:],
                    func=AF.Ln,
                    bias=tiny_t[0:2],
                )
                hqB = auxh_pool.tile([128, 512], F32, tag="auxh", name="hqB")
                nc.tensor.matmul(
                    hqB[:], bc2_sb[:], hl2[:], start=True, stop=True
                )
                hqB_sb = hpool.tile([128, 512], F16, tag="hqb")
                nc.scalar.activation(
                    out=hqB_sb[:], in_=hqB[:], func=AF.Exp, bias=0.0,
                    scale=-0.5,
                )
                nc.vector.tensor_mul(qn[:, ssl], q_raw[:], hqB_sb[:])

            def rotary_finish(qn, dst, kfac=False):
                # rotate-half swap within each head (partition +-32)
                qsw = cpool.tile([128, N], F16, tag="qsw", bufs=2)
                for h0 in (0, 64):
                    nc.sync.dma_start(
                        out=qsw[h0 : h0 + 32, :], in_=qn[h0 + 32 : h0 + 64, :]
                    )
                    nc.sync.dma_start(
                        out=qsw[h0 + 32 : h0 + 64, :], in_=qn[h0 : h0 + 32, :]
                    )
                nc.vector.tensor_mul(qn[:], qn[:], cos_sb[:])
                nc.vector.tensor_mul(qsw[:], qsw[:], sin_sb[:])
                if not kfac:
                    nc.vector.tensor_add(dst[:], qn[:], qsw[:])
                    return
                nc.vector.tensor_add(qn[:], qn[:], qsw[:])
                # fold the 8/||k|| per-token factors into kr, strip by strip
                for s in range(4):
                    ssl = slice(s * 512, (s + 1) * 512)
                    khB = auxh_pool.tile(
                        [128, 512], F32, tag="auxh", name="khB"
                    )
                    nc.tensor.matmul(
                        khB[:], bc2_sb[:], hkl2[:, ssl], start=True, stop=True
                    )
                    khB_sb = hpool.tile([128, 512], F16, tag="hqb")
                    nc.scalar.activation(
                        out=khB_sb[:], in_=khB[:], func=AF.Exp, bias=0.0,
                        scale=-0.5,
                    )
                    nc.vector.tensor_mul(dst[:, ssl], qn[:, ssl], khB_sb[:])

            # ---- prologue: token strips, K and V projections ----
            qn_k = cpool.tile([128, N], F16, tag="qn", bufs=2, name="qn_k")
            for s in range(4):
                for tt in range(4 * s, 4 * s + 4):
                    a_tile(tt)
                k_slice(s)
                for tb in range(4 * s, 4 * s + 4):
                    v_tile(tb)
                if s == 2:
                    nc.sync.dma_start(
                        out=wq_sb[:],
                        in_=t_wq.rearrange("(c p) q -> p c q", p=128),
                    )
            # Q pair 0, then rotaries (k gets the 8/||k|| factor folded in)
            qn_q0 = cpool.tile([128, N], F16, tag="qn", bufs=2, name="qn_q0")
            for s in range(4):
                q_slice(0, s, qn_q0)
            rotary_finish(qn_k, kr_t, kfac=True)
            rotary_finish(qn_q0, qr_t[0])

            # ---- attention windows W0..W2: attention(p) + Q(p+1) chunks ----
            def attention(p, filler):
                """Attention for pair p; filler(j) emits one PE filler unit
                after chunk-block j (j in 0..63) to plug exp-wait bubbles.
                The AV matmuls run one chunk behind S/exp so the in-order PE
                queue never puts S(c+1) behind an exp(c) wait."""
                for st in range(NSTRIP):
                    ssl = slice(st * STRIP, (st + 1) * STRIP)
                    av = [
                        av_pool.tile(
                            [128, 4, DH + 1], F32, tag=f"av{h}", name=f"av{h}"
                        )
                        for h in (0, 1)
                    ]
                    et_prev = None
                    for c in range(NC + 1):
                        if c < NC:
                            ck = slice(c * 128, (c + 1) * 128)
                            sps = sps_pool.tile(
                                [128, 2, STRIP], F32, tag="sps"
                            )
                            nc.tensor.matmul(
                                sps[:, 0, :],
                                kr_t[0:DH, ck],
                                qr_t[p][0:DH, ssl],
                                start=True,
                                stop=True,
                            )
                            nc.tensor.matmul(
                                sps[:, 1, :],
                                kr_t[DH:128, ck],
                                qr_t[p][DH:128, ssl],
                                start=True,
                                stop=True,
                            )
                            et = epool.tile([128, 2, STRIP], F16, tag="et")
                            nc.scalar.activation(
                                out=et[:], in_=sps[:], func=AF.Exp, bias=0.0
                            )
                        if c > 0:
                            cp = c - 1
                            # PSUM zero-regions are bank-granular: start only
                            # on the bank's first write, stop on its last;
                            # per-byte zero-on-first-write keeps the four
                            # q-subtile accumulations independent.
                            for h in (0, 1):
                                for qt in range(4):
                                    nc.tensor.matmul(
                                        av[h][:, qt, :],
                                        et_prev[:, h, qt * 128 : (qt + 1) * 128],
                                        v_all[:, cp, h, :],
                                        start=(cp == 0 and qt == 0),
                                        stop=(cp == NC - 1 and qt == 3),
                                        skip_group_check=True,
                                    )
                            filler(st * NC + cp)
                        et_prev = et if c < NC else None
                    # normalize: per-partition 1/denominator, then xbar to o_t
                    recs = []
                    for h in (0, 1):
                        rec = dpool.tile(
                            [128, 4, 1], F32, tag=f"rec{h}", name="rec"
                        )
                        nc.vector.reciprocal(
                            out=rec[:], in_=av[h][:, :, DH : DH + 1]
                        )
                        recs.append(rec)
                    for qt in range(4):
                        tb = st * 4 + qt
                        on = onat_pool.tile(
                            [128, 128], F16, tag="onat", name="on"
                        )
                        for h in (0, 1):
                            nc.vector.tensor_scalar_mul(
                                on[:, h * DH : (h + 1) * DH],
                                av[h][:, qt, 0:DH],
                                recs[h][:, qt, :],
                            )
                        nc.sync.dma_start_transpose(
                            o_t[p][:, tb * 128 : (tb + 1) * 128], on[:]
                        )

            for p in range(3):
                # Q(p+1) projection matmuls, one per attention chunk-block:
                # slice s uses blocks 16s..16s+15, then its norm tail.
                qps_live = [None]

                def filler(j, p=p, qps_live=qps_live):
                    s, c = j // NC, j % NC
                    if c == 0:
                        qps_live[0] = auxq_pool.tile(
                            [128, 512], F32, tag="auxq", name="qps"
                        )
                    nc.tensor.matmul(
                        qps_live[0][:],
                        wq_sb[:, c, (p + 1) * 128 : (p + 2) * 128],
                        xt_s[s][:, c, :],
                        start=(c == 0),
                        stop=(c == NC - 1),
                    )
                    if c == NC - 1:
                        q_slice_norm(p + 1, s, qn_q, qps_live[0])

                qn_q = cpool.tile([128, N], F16, tag="qn", bufs=2, name="qn_q")
                attention(p, filler)
                rotary_finish(qn_q, qr_t[p + 1])

        # ---------------- W3: attention(3) + out-projection ----------------
        with ExitStack() as bb:
            wo_pool = bb.enter_context(tc.tile_pool(name="wop", bufs=1))
            outp = bb.enter_context(tc.tile_pool(name="outp", bufs=2))
            wo_sb = wo_pool.tile([128, NPAIR, DIM], F16)
            nc.sync.dma_start(
                out=wo_sb[:], in_=t_wo.rearrange("(p o) d -> o p d", o=128)
            )
            osb = [None]

            def out_filler(j):
                # one 512-col out-proj slice (4 accum matmuls) per chunk;
                # tile tb is ready once attention(3) strip tb//4 is done, so
                # run strip st's chunks against strip st-1's tiles.
                st, c = j // NC, j % NC
                tb = (st - 1) * 4 + c // 4
                cs = c % 4
                if st == 0:
                    return
                _out_tile_slice(tb, cs)

            def _out_tile_slice(tb, cs):
                if cs == 0:
                    osb[0] = outp.tile([128, DIM], F16, tag="osb", name="osb")
                xpool = auxq_pool if cs % 2 == 0 else auxh_pool
                xps = xpool.tile(
                    [128, 512], F32, tag=xpool is auxq_pool and "auxq" or "auxh",
                    name="xps",
                )
                for pp in range(NPAIR):
                    nc.tensor.matmul(
                        xps[:],
                        o_t[pp][:, tb * 128 : (tb + 1) * 128],
                        wo_sb[:, pp, cs * 512 : (cs + 1) * 512],
                        start=(pp == 0),
                        stop=(pp == NPAIR - 1),
                    )
                nc.vector.tensor_copy(
                    osb[0][:, cs * 512 : (cs + 1) * 512], xps[:]
                )
                if cs == 3:
                    nc.sync.dma_start(
                        out=t_out[tb * 128 : (tb + 1) * 128, :], in_=osb[0][:]
                    )

            attention(3, out_filler)
            for tb in range(12, 16):
                for cs in range(4):
                    _out_tile_slice(tb, cs)

    nc.compile()
    return nc


def _core_inputs(core, tokens, rotary, norm_w, Wq, Wkv, Wo, gamma_q, gamma_k):
    b, g = core // TPG, core % TPG
    # pair order: pair p = (8g+p, 8g+4+p); lo half -> kv head 2g, hi -> 2g+1
    heads = []
    for p in range(NPAIR):
        heads += [QH_PER * g + p, QH_PER * g + NPAIR + p]
    cols = np.concatenate([np.arange(h * DH, (h + 1) * DH) for h in heads])

    nw = norm_w[:, None].astype(np.float32)
    gq = (gamma_q.astype(np.float32) + 1.0)  # [32, 64]
    gk = (gamma_k.astype(np.float32) + 1.0)  # [8, 64]
    gq_cols = np.concatenate([gq[h] for h in heads])  # [512]
    kheads = [2 * g, 2 * g + 1]
    gk_cols = np.concatenate([gk[h] for h in kheads])  # [128]

    wq = (Wq * nw)[:, cols] * gq_cols[None, :]
    kcols = np.arange(2 * g * DH, (2 * g + 2) * DH)
    wk = (Wkv[:, : KVH * DH] * nw)[:, kcols] * gk_cols[None, :]
    wv = (Wkv[:, KVH * DH :] * nw)[:, kcols]
    wo = Wo[cols, :]

    # hsum selectors: per-dim 1/G^2 so ||q||^2 is of the raw projection
    selq = np.zeros((128, 2 * NPAIR), np.float16)
    selk = np.zeros((128, 2), np.float16)
    for p in range(NPAIR):
        selq[:DH, 2 * p] = 1.0 / np.square(gq[heads[2 * p]])
        selq[DH:, 2 * p + 1] = 1.0 / np.square(gq[heads[2 * p + 1]])
    selk[:DH, 0] = 1.0 / np.square(gk[kheads[0]])
    selk[DH:, 1] = 1.0 / np.square(gk[kheads[1]])
    bc2 = np.zeros((2, 128), np.float32)
    bc2[0, :DH] = 1.0
    bc2[1, DH:] = 1.0

    cosT = np.cos(rotary).T.astype(np.float32)  # [64, N]
    sinT = np.sin(rotary).T.astype(np.float32)
    sinS = np.concatenate([-sinT[:32], sinT[32:]], axis=0)
    cos_t = np.tile(cosT, (2, 1)).astype(np.float16)
    sins_t = np.tile(sinS, (2, 1)).astype(np.float16)

    return {
        "tokens": np.ascontiguousarray(tokens[b]).astype(np.float16),
        "wq": np.ascontiguousarray(wq.astype(np.float16)),
        "wk": np.ascontiguousarray(wk.astype(np.float16)),
        "wv": np.ascontiguousarray(wv.astype(np.float16)),
        "wo": np.ascontiguousarray(wo.astype(np.float16)),
        "selq": selq,
        "selk": selk,
        "bc2": bc2,
        "cos_t": cos_t,
        "sins_t": sins_t,
    }


def kernel(tokens, rotary, norm_w, Wq, Wkv, Wo, gamma_q, gamma_k, _bench=None):
    if "nc" not in _BUILD:
        _BUILD["nc"] = _build_nc()
    nc = _BUILD["nc"]

    in_maps = [
        _core_inputs(c, tokens, rotary, norm_w, Wq, Wkv, Wo, gamma_q, gamma_k)
        for c in range(NCORES)
    ]
    kw = dict(_bench or {})
    res = run_bass_kernel_spmd(nc, in_maps, list(range(NCORES)), **kw)
    if _bench is not None:
        _BUILD["last"] = res

    out = np.empty((B, N, DIM), np.float32)
    for b in range(B):
        acc = res.results[b * TPG]["out"].astype(np.float32)
        for g in range(1, TPG):
            acc = acc + res.results[b * TPG + g]["out"].astype(np.float32)
        out[b] = acc
    return out
